# revision 6
# baseline (speedup 1.0000x reference)
"""CARAFE++ content-aware upsampling kernel for Trainium2 (8 NeuronCores).

Problem: x (4, 256, 64, 64) f32; 1x1 compress conv (256->64) + relu;
3x3 encoder conv (64->100); softmax over 25 taps; content-aware reassembly
(5x5 dynamic per-pixel filter, scale 2); flat pixel rearrangement to
(4, 256, 128, 128).

Sharding: 8 cores = 4 batches x 2 row-halves (32 rows each + halo).
All compute per-core independent (no collectives).

Per-core pipeline (v2):
  - host sends x as fp16 in BOTH layouts: (ch, px) for the convs and
    pre-transposed (px, ch) for the reassembly (kills 36 PE transposes)
  - conv1 as 2 accumulated fp16 matmuls per col-tile; relu eviction written
    twice (partitions 0-63 = feat, 64-127 = feat shifted one row) so conv2
    can pair vertical taps into K=128 matmuls (9 -> 6 matmuls per tile)
  - conv2 + bias + exp -> wk fp16; DMA-XBAR transpose -> wkT (px-major)
  - per 128-px block: DVE tap-group sums + reciprocal + normalize (softmax),
    gpsimd local_scatter builds band-matrix-transpose S^T, one DMA-XBAR
    transpose turns it into 12 [128,128] S panels (src-major)
  - reassembly out = x_T.T @ S: 6 accumulated fp16 matmuls per block
  - interleaved fp16 eviction; batched DMA store; fp16 -> f32 on host
"""
import sys

sys.path.insert(0, "/opt/trn_rl_repo")

import numpy as np
from contextlib import ExitStack

import concourse.bass as bass
import concourse.bacc as bacc
import concourse.tile as tile
from concourse import mybir
from concourse.bass_utils import run_bass_kernel_spmd

B, C, H, W = 4, 256, 64, 64
SCALE, K, COMP, G = 2, 5, 4, 1
MID = 64
ENC = 100          # K*K*SCALE*SCALE
NROW = 36          # x rows per core (32 + 2 halo each side)
NPX = NROW * W     # 2304
FROW = 34          # feat rows (rows r0-1 .. r0+32)
FPW = W + 2        # 66, feat row W-padded
NBLK = 16          # output row-pair blocks per core
NJB = 18           # x row-pair blocks per core

f32 = mybir.dt.float32
f16 = mybir.dt.float16
i16 = mybir.dt.int16

_CACHE = {}

# x chunk column ranges (per 128-ch half): conv1 tile nt reads x cols
# 64+512*nt .. 64+512*(nt+1); chunk c covers tiles CHUNK_TILES[c]
CHUNKS = [(0, 1088), (1088, 2112), (2112, 2304)]
TILE_CHUNK = [0, 0, 1, 1, 2]


def _build_idxs():
    """Per-partition scatter indices encoding the CARAFE tap geometry.

    Partition = out-pixel (rt, w) within a row-pair block. Slot = (p, dy, dx)
    = wk channel order. Value = position in the (p, jb_rel, rb, wi) scatter
    destination, or -1 when the tap falls outside the image in W.
    """
    idxs = np.full((128, 100), -1, np.int16)
    for rt in range(2):
        for w in range(W):
            part = rt * W + w
            for p in range(4):
                for dy in range(-2, 3):
                    jb_rel = (rt + dy + 2) // 2      # 0..2
                    rb = (rt + dy) % 2
                    for dx in range(-2, 3):
                        wi = w + dx
                        if 0 <= wi < W:
                            slot = p * 25 + (dy + 2) * 5 + (dx + 2)
                            idxs[part, slot] = p * 384 + jb_rel * 128 + rb * 64 + wi
    return idxs


def _build_nc():
    nc = bacc.Bacc("TRN2", target_bir_lowering=False, debug=False, num_devices=8)

    # ---- DRAM I/O (per-core shapes)
    d_x = nc.dram_tensor("x", [C, NPX], f16, kind="ExternalInput")
    d_xt = nc.dram_tensor("xt", [128, NJB * 256], f16, kind="ExternalInput")
    d_wc = nc.dram_tensor("wc", [C, MID], f16, kind="ExternalInput")        # W_comp.T
    d_wep = nc.dram_tensor("wep", [128, 3 * ENC], f16, kind="ExternalInput")  # taps (0,j)|(1,j)
    d_wes = nc.dram_tensor("wes", [MID, 3 * ENC], f16, kind="ExternalInput")  # taps (2,j)
    d_bc = nc.dram_tensor("bc", [MID, 1], f32, kind="ExternalInput")
    d_be = nc.dram_tensor("be", [ENC, 1], f32, kind="ExternalInput")
    d_idx = nc.dram_tensor("idx", [128, 100], i16, kind="ExternalInput")
    d_out = nc.dram_tensor("out", [C, 32 * 256], f16, kind="ExternalOutput")

    with tile.TileContext(nc) as tc, ExitStack() as ctx:
        sb1 = ctx.enter_context(tc.tile_pool(name="sb1", bufs=1))
        sbw = ctx.enter_context(tc.tile_pool(name="sbw", bufs=2))
        ps = ctx.enter_context(tc.tile_pool(name="ps", bufs=2, space="PSUM"))

        # ---- load weights / constants (SP queue), bulk tensors (Act queue)
        wc0 = sb1.tile([128, MID], f16, tag="wc0")
        wc1 = sb1.tile([128, MID], f16, tag="wc1")
        nc.sync.dma_start(out=wc0, in_=d_wc[0:128, :])
        nc.sync.dma_start(out=wc1, in_=d_wc[128:256, :])
        bc = sb1.tile([MID, 1], f32, tag="bc")
        nc.sync.dma_start(out=bc, in_=d_bc[:])

        xt = sb1.tile([128, NJB * 256], f16, tag="xt")
        nc.scalar.dma_start(out=xt, in_=d_xt[:])
        sidx = sb1.tile([128, 100], i16, tag="sidx")
        nc.scalar.dma_start(out=sidx, in_=d_idx[:])
        be = sb1.tile([ENC, 1], f32, tag="be")
        nc.scalar.dma_start(out=be, in_=d_be[:])

        # x chunks (both 128-ch halves per chunk), interleaved for early conv1
        xc = [[None] * len(CHUNKS) for _ in range(2)]
        for k, (c0, c1) in enumerate(CHUNKS):
            for h in range(2):
                t = sbw.tile([128, c1 - c0], f16, tag=f"xc{h}{k}", bufs=1)
                nc.sync.dma_start(out=t, in_=d_x[h * 128:(h + 1) * 128, c0:c1])
                xc[h][k] = t
        wep = sb1.tile([128, 3, ENC], f16, tag="wep")
        nc.sync.dma_start(out=wep, in_=d_wep[:].rearrange("m (t o) -> m t o", t=3))
        wes = sb1.tile([MID, 3, ENC], f16, tag="wes")
        nc.sync.dma_start(out=wes, in_=d_wes[:].rearrange("m (t o) -> m t o", t=3))

        # ---- feat (W-padded, fp16): partitions 0-63 = feat rows r,
        # partitions 64-127 = feat rows r+1 (so conv2 vertical tap pairs
        # contract with K=128). Zero only the pad columns (0 and 65).
        featD = sb1.tile([128, FROW * FPW], f16, tag="featD")
        pad = bass.AP(tensor=featD.tensor, offset=featD.offset,
                      ap=[featD.ap[0], [FPW, FROW], [FPW - 1, 2]])
        nc.vector.memset(pad, 0.0)

        # ---- conv1 (1x1, 256->64) + relu -> featD (both halves)
        lo64 = featD[0:64, :]
        for nt in range(5):
            n0 = W + nt * 512          # px offset into x
            n = min(512, 2240 - n0)
            ck = TILE_CHUNK[nt]
            lo = n0 - CHUNKS[ck][0]
            pf = ps.tile([MID, 512], f32, tag="pf")
            nc.tensor.matmul(pf[:, :n], wc0[:], xc[0][ck][:, lo:lo + n],
                             start=True, stop=False)
            nc.tensor.matmul(pf[:, :n], wc1[:], xc[1][ck][:, lo:lo + n],
                             start=False, stop=True)
            fp0 = n0 // W - 1
            nrows = n // W
            dst = bass.AP(
                tensor=lo64.tensor, offset=lo64.offset + fp0 * FPW + 1,
                ap=[lo64.ap[0], [FPW, nrows], [1, W]],
            )
            nc.scalar.activation(out=dst, in_=pf[:, :n].rearrange("m (r w) -> m r w", w=W),
                                 func=mybir.ActivationFunctionType.Relu,
                                 bias=bc[:], scale=1.0)
            # duplicate into partitions 64-127, shifted one row down
            hi = featD[64:128, :]
            if nt == 0:
                dstB = bass.AP(tensor=hi.tensor, offset=hi.offset + 1,
                               ap=[hi.ap[0], [FPW, nrows - 1], [1, W]])
                srcB = pf[:, W:n].rearrange("m (r w) -> m r w", w=W)
            else:
                dstB = bass.AP(tensor=hi.tensor, offset=hi.offset + (fp0 - 1) * FPW + 1,
                               ap=[hi.ap[0], [FPW, nrows], [1, W]])
                srcB = pf[:, :n].rearrange("m (r w) -> m r w", w=W)
            nc.scalar.activation(out=dstB, in_=srcB,
                                 func=mybir.ActivationFunctionType.Relu,
                                 bias=bc[:], scale=1.0)

        # ---- conv2 (3x3, 64->100) + bias + exp -> wk (fp16), then
        # DMA-XBAR transpose -> wkT[q] [128px, 4, 112] (cols 100-111 junk)
        wkT = []
        for q in range(4):
            h0 = q * 8
            pw = ps.tile([ENC, 512], f32, tag="pw")
            for j in range(3):
                rhs = bass.AP(tensor=featD.tensor,
                              offset=featD.offset + h0 * FPW + j,
                              ap=[featD.ap[0], [FPW, 8], [1, W]])
                nc.tensor.matmul(pw[:], wep[:, j, :], rhs,
                                 start=(j == 0), stop=False)
            for j in range(3):
                rhs = bass.AP(tensor=lo64.tensor,
                              offset=lo64.offset + (h0 + 2) * FPW + j,
                              ap=[lo64.ap[0], [FPW, 8], [1, W]])
                nc.tensor.matmul(pw[:], wes[:, j, :], rhs,
                                 start=False, stop=(j == 2))
            wkq = sbw.tile([112, 512], f16, tag="wkq", bufs=2)
            nc.scalar.activation(out=wkq[0:100, :], in_=pw[:],
                                 func=mybir.ActivationFunctionType.Exp,
                                 bias=be[:], scale=1.0)
            wt = sb1.tile([128, 4, 112], f16, tag=f"wkT{q}")
            nc.sync.dma_start(out=wt, in_=wkq[:], transpose=True)
            wkT.append(wt)

        # ---- per-block: softmax-normalize, scatter, DMA-transpose, reassemble
        for g in range(4):                     # output DMA groups of 4 blocks
            ostg0 = sbw.tile([128, 2048], f16, tag="ostg0", bufs=2)
            ostg1 = sbw.tile([128, 2048], f16, tag="ostg1", bufs=2)
            ostg = [ostg0, ostg1]
            for i in range(4):
                t = g * 4 + i
                q, qi = t // 4, t % 4
                wkTt = wkT[q]
                # softmax over 25 taps: group sums, reciprocal, scale
                sums = sbw.tile([128, 4], f32, tag="sums", bufs=2)
                nc.vector.tensor_reduce(
                    out=sums[:],
                    in_=wkTt[:, qi, 0:100].rearrange("z (p k) -> z p k", k=25),
                    axis=mybir.AxisListType.X, op=mybir.AluOpType.add)
                recip = sbw.tile([128, 4], f32, tag="recip", bufs=2)
                nc.vector.reciprocal(recip[:], sums[:])
                wkN = sbw.tile([128, 100], f16, tag="wkN", bufs=3)
                rb = bass.AP(tensor=recip.tensor, offset=recip.offset,
                             ap=[recip.ap[0], [1, 4], [0, 25]])
                nc.vector.tensor_mul(
                    wkN[:].rearrange("z (p k) -> z p k", k=25),
                    wkTt[:, qi, 0:100].rearrange("z (p k) -> z p k", k=25),
                    rb)

                # scatter into band-matrix transpose layout (p, jb_rel, rb, wi)
                sdst = sbw.tile([128, 1536], f16, tag="sdst", bufs=3)
                nc.gpsimd.local_scatter(
                    out_ap=sdst[:], data_ap=wkN[:], idxs_ap=sidx[:],
                    channels=128, num_elems=1536, num_idxs=100)

                # DMA-XBAR transpose: all 12 [128,128] S panels in one shot
                s16 = sbw.tile([128, 12, 128], f16, tag="s16", bufs=3)
                nc.sync.dma_start(out=s16, in_=sdst[:], transpose=True)

                # reassembly: po[ch, (p, outpx)] += xt.T @ S
                for ch in range(2):
                    po = ps.tile([128, 512], f32, tag="po", bufs=3)
                    for dj in range(3):
                        rhs = bass.AP(tensor=s16.tensor,
                                      offset=s16.offset + dj * 128,
                                      ap=[s16.ap[0], [384, 4], [1, 128]])
                        nc.tensor.matmul(
                            po[:],
                            xt[:, (t + dj) * 256 + ch * 128:
                               (t + dj) * 256 + ch * 128 + 128],
                            rhs, start=(dj == 0), stop=(dj == 2))
                    # evict with (p, rt, w) -> (rt, w, p) interleave, fp16
                    src = bass.AP(tensor=po.tensor, offset=po.offset,
                                  ap=[po.ap[0], [64, 2], [1, 64], [128, 4]])
                    nc.scalar.activation(
                        out=ostg[ch][:, i * 512:(i + 1) * 512].rearrange(
                            "c (a b d) -> c a b d", a=2, b=64),
                        in_=src,
                        func=mybir.ActivationFunctionType.Copy, scale=1.0)
            for ch in range(2):
                nc.scalar.dma_start(
                    out=d_out[ch * 128:(ch + 1) * 128, g * 2048:(g + 1) * 2048],
                    in_=ostg[ch])

    nc.compile()
    return nc


def _host_prep(x, W_comp, b_comp, W_enc, b_enc):
    """Build per-core input maps."""
    idxs = _build_idxs()
    wcT = np.ascontiguousarray(W_comp.T).astype(np.float16)            # (256, 64)
    # weT[m, tap, o] = W_enc[o, m, i, j], tap = 3i + j
    weT = np.ascontiguousarray(W_enc.transpose(1, 2, 3, 0)).astype(np.float16)
    wep = np.empty((128, 3, ENC), np.float16)
    wep[0:64] = weT[:, 0, :, :]
    wep[64:128] = weT[:, 1, :, :]
    wep = np.ascontiguousarray(wep.reshape(128, 3 * ENC))
    wes = np.ascontiguousarray(weT[:, 2, :, :].reshape(MID, 3 * ENC))
    bc = np.ascontiguousarray(b_comp.reshape(MID, 1)).astype(np.float32)
    be = np.ascontiguousarray(b_enc.reshape(ENC, 1)).astype(np.float32)

    xp = np.pad(x, ((0, 0), (0, 0), (2, 2), (0, 0)))   # (B, C, 68, 64)
    in_maps = []
    for core in range(8):
        b, half = core // 2, core % 2
        r0 = 32 * half
        xs = np.ascontiguousarray(
            xp[b, :, r0:r0 + NROW, :].reshape(C, NPX)).astype(np.float16)
        xth = np.ascontiguousarray(
            xs.reshape(C, NJB, 128).transpose(2, 1, 0)).reshape(128, NJB * C)
        in_maps.append(dict(x=xs, xt=xth, wc=wcT, wep=wep, wes=wes,
                            bc=bc, be=be, idx=idxs))
    return in_maps


def kernel(x, W_comp, b_comp, W_enc, b_enc):
    x = np.asarray(x, np.float32)
    W_comp = np.asarray(W_comp, np.float32)
    b_comp = np.asarray(b_comp, np.float32)
    W_enc = np.asarray(W_enc, np.float32)
    b_enc = np.asarray(b_enc, np.float32)

    if "nc" not in _CACHE:
        _CACHE["nc"] = _build_nc()
    nc = _CACHE["nc"]

    in_maps = _host_prep(x, W_comp, b_comp, W_enc, b_enc)
    res = run_bass_kernel_spmd(nc, in_maps, core_ids=list(range(8)))

    out = np.empty((B, C, 128, 128), np.float32)
    for core in range(8):
        b, half = core // 2, core % 2
        seg = res.results[core]["out"].astype(np.float32)   # (256, 8192)
        out[b, :, 64 * half:64 * (half + 1), :] = seg.reshape(C, 64, 128)
    return out


if __name__ == "__main__":
    rng = np.random.default_rng(0)
    x = rng.standard_normal((B, C, H, W)).astype(np.float32)
    W_comp = (rng.standard_normal((MID, C)) / np.sqrt(C)).astype(np.float32)
    b_comp = np.zeros((MID,), np.float32)
    W_enc = (rng.standard_normal((ENC, MID, 3, 3)) / np.sqrt(MID * 9)).astype(np.float32)
    b_enc = np.zeros((ENC,), np.float32)
    out = kernel(x, W_comp, b_comp, W_enc, b_enc)
    print("out", out.shape, out.dtype, float(np.abs(out).mean()))


# revision 33
# speedup vs baseline: 1.1119x; 1.1119x over previous
"""CARAFE++ content-aware upsampling kernel for Trainium2 (8 NeuronCores).

Problem: x (4, 256, 64, 64) f32; 1x1 compress conv (256->64) + relu;
3x3 encoder conv (64->100); softmax over 25 taps; content-aware reassembly
(5x5 dynamic per-pixel filter, scale 2); flat pixel rearrangement to
(4, 256, 128, 128).

Sharding: 8 cores = 4 batches x 2 row-halves (32 rows each + halo).
All compute per-core independent (no collectives).

Per-core pipeline (v2):
  - host sends x as fp16 in BOTH layouts: (ch, px) for the convs and
    pre-transposed (px, ch) for the reassembly (kills 36 PE transposes)
  - conv1 as 2 accumulated fp16 matmuls per col-tile; relu eviction written
    twice (partitions 0-63 = feat, 64-127 = feat shifted one row) so conv2
    can pair vertical taps into K=128 matmuls (9 -> 6 matmuls per tile)
  - conv2 + bias + exp -> wk fp16; DMA-XBAR transpose -> wkT (px-major)
  - per 128-px block: DVE tap-group sums + reciprocal + normalize (softmax),
    gpsimd local_scatter builds band-matrix-transpose S^T, one DMA-XBAR
    transpose turns it into 12 [128,128] S panels (src-major)
  - reassembly out = x_T.T @ S: 6 accumulated fp16 matmuls per block
  - interleaved fp16 eviction; batched DMA store; fp16 -> f32 on host
"""
import sys

sys.path.insert(0, "/opt/trn_rl_repo")

import numpy as np
from contextlib import ExitStack

import concourse.bass as bass
import concourse.bacc as bacc
import concourse.tile as tile
from concourse import mybir
from concourse.bass_utils import run_bass_kernel_spmd

B, C, H, W = 4, 256, 64, 64
SCALE, K, COMP, G = 2, 5, 4, 1
MID = 64
ENC = 100          # K*K*SCALE*SCALE
NROW = 36          # x rows per core (32 + 2 halo each side)
NPX = NROW * W     # 2304
FROW = 34          # feat rows (rows r0-1 .. r0+32)
FPW = W + 2        # 66, feat row W-padded
NBLK = 16          # output row-pair blocks per core
NJB = 18           # x row-pair blocks per core

f32 = mybir.dt.float32
f16 = mybir.dt.float16
i16 = mybir.dt.int16

_CACHE = {}

# x chunk column ranges (per 128-ch half): conv1 tile nt reads x cols
# 64+512*nt .. 64+512*(nt+1); chunk c covers tiles CHUNK_TILES[c]
CHUNKS = [(0, 1088), (1088, 2112), (2112, 2304)]
TILE_CHUNK = [0, 0, 1, 1, 2]


def _build_idxs():
    """Per-partition scatter indices encoding the CARAFE tap geometry.

    Partition = out-pixel (rt, w) within a row-pair block. Slot = (p, dy, dx)
    = wk channel order. Value = position in the (p, jb_rel, rb, wi) scatter
    destination, or -1 when the tap falls outside the image in W.
    """
    idxs = np.full((128, 100), -1, np.int16)
    for rt in range(2):
        for w in range(W):
            part = rt * W + w
            for p in range(4):
                for dy in range(-2, 3):
                    jb_rel = (rt + dy + 2) // 2      # 0..2
                    rb = (rt + dy) % 2
                    for dx in range(-2, 3):
                        wi = w + dx
                        if 0 <= wi < W:
                            slot = p * 25 + (dy + 2) * 5 + (dx + 2)
                            idxs[part, slot] = p * 384 + jb_rel * 128 + rb * 64 + wi
    return idxs


def _build_nc():
    nc = bacc.Bacc("TRN2", target_bir_lowering=False, debug=False, num_devices=8)

    # ---- DRAM I/O (per-core shapes)
    d_x = nc.dram_tensor("x", [C, NPX], f16, kind="ExternalInput")
    d_xt = nc.dram_tensor("xt", [128, NJB * 256], f16, kind="ExternalInput")
    d_wc = nc.dram_tensor("wc", [C, MID], f16, kind="ExternalInput")        # W_comp.T
    d_we = nc.dram_tensor("we", [MID, 9 * ENC], f16, kind="ExternalInput")
    d_bc = nc.dram_tensor("bc", [MID, 1], f32, kind="ExternalInput")
    d_be = nc.dram_tensor("be", [ENC, 1], f32, kind="ExternalInput")
    d_idx = nc.dram_tensor("idx", [128, 100], i16, kind="ExternalInput")
    d_out = nc.dram_tensor("out", [C, 32 * 256], f16, kind="ExternalOutput")

    with tile.TileContext(nc) as tc, ExitStack() as ctx:
        sb1 = ctx.enter_context(tc.tile_pool(name="sb1", bufs=1))
        sbw = ctx.enter_context(tc.tile_pool(name="sbw", bufs=2))
        ps = ctx.enter_context(tc.tile_pool(name="ps", bufs=2, space="PSUM"))

        # ---- load weights / constants (SP queue), bulk tensors (Act queue)
        wc0 = sb1.tile([128, MID], f16, tag="wc0")
        wc1 = sb1.tile([128, MID], f16, tag="wc1")
        nc.sync.dma_start(out=wc0, in_=d_wc[0:128, :])
        nc.sync.dma_start(out=wc1, in_=d_wc[128:256, :])
        bc = sb1.tile([MID, 1], f32, tag="bc")
        nc.sync.dma_start(out=bc, in_=d_bc[:])

        we = sb1.tile([MID, 9, ENC], f16, tag="we")
        nc.scalar.dma_start(out=we, in_=d_we[:].rearrange("m (t o) -> m t o", t=9))
        be = sb1.tile([ENC, 1], f32, tag="be")
        nc.scalar.dma_start(out=be, in_=d_be[:])
        sidx = sb1.tile([128, 100], i16, tag="sidx")
        nc.scalar.dma_start(out=sidx, in_=d_idx[:])
        xt = sb1.tile([128, NJB * 256], f16, tag="xt")
        nc.scalar.dma_start(out=xt, in_=d_xt[:])

        # identity for PE transposes
        ident = sb1.tile([128, 128], f16, tag="ident")
        nc.vector.memset(ident, 1.0)
        nc.gpsimd.affine_select(
            out=ident[:], in_=ident[:], pattern=[[-1, 128]], base=0,
            channel_multiplier=1, compare_op=mybir.AluOpType.is_equal, fill=0.0,
        )

        # x chunks (both 128-ch halves per chunk), interleaved for early conv1
        xc = [[None] * len(CHUNKS) for _ in range(2)]
        for k, (c0, c1) in enumerate(CHUNKS):
            for h in range(2):
                t = sbw.tile([128, c1 - c0], f16, tag=f"xc{h}{k}", bufs=1)
                nc.sync.dma_start(out=t, in_=d_x[h * 128:(h + 1) * 128, c0:c1])
                xc[h][k] = t
        # ---- feat (W-padded, fp16) on partitions 0-63; zero pad cols
        featD = sb1.tile([MID, FROW * FPW], f16, tag="featD")
        pad = bass.AP(tensor=featD.tensor, offset=featD.offset,
                      ap=[featD.ap[0], [FPW, FROW], [FPW - 1, 2]])
        nc.vector.memset(pad, 0.0)

        # ---- conv1 (1x1, 256->64) + relu -> featD rows, interleaved with
        # conv2 (3x3, 64->100, 9 shifted accumulated matmuls) + bias + exp
        # -> wkq[q] [100, 512] fp16.
        wkq = [None] * 4

        def conv1_tile(nt):
            n0 = W + nt * 512          # px offset into x
            n = min(512, 2240 - n0)
            ck = TILE_CHUNK[nt]
            lo = n0 - CHUNKS[ck][0]
            pf = ps.tile([MID, 512], f32, tag="pf", bufs=1, name=f"pf{nt}")
            nc.tensor.matmul(pf[:, :n], wc0[:], xc[0][ck][:, lo:lo + n],
                             start=True, stop=False)
            nc.tensor.matmul(pf[:, :n], wc1[:], xc[1][ck][:, lo:lo + n],
                             start=False, stop=True)
            fp0 = n0 // W - 1
            nrows = n // W
            dst = bass.AP(
                tensor=featD.tensor, offset=featD.offset + fp0 * FPW + 1,
                ap=[featD.ap[0], [FPW, nrows], [1, W]],
            )
            nc.scalar.activation(out=dst, in_=pf[:, :n].rearrange("m (r w) -> m r w", w=W),
                                 func=mybir.ActivationFunctionType.Relu,
                                 bias=bc[:], scale=1.0)

        def conv2_tile(q):
            h0 = q * 8
            pw = ps.tile([ENC, 512], f32, tag="pw", name=f"pw{q}")
            for tap in range(9):
                i, j = tap // 3, tap % 3
                rhs = bass.AP(tensor=featD.tensor,
                              offset=featD.offset + (h0 + i) * FPW + j,
                              ap=[featD.ap[0], [FPW, 8], [1, W]])
                nc.tensor.matmul(pw[:], we[:, tap, :], rhs,
                                 start=(tap == 0), stop=(tap == 8))
            wt = sbw.tile([ENC, 512], f16, tag=f"wkq{q}", name=f"wkq{q}")
            nc.scalar.activation(out=wt[:], in_=pw[:],
                                 func=mybir.ActivationFunctionType.Exp,
                                 bias=be[:], scale=1.0)
            wkq[q] = wt

        conv1_tile(0)
        conv1_tile(1)
        conv2_tile(0)
        conv1_tile(2)
        conv2_tile(1)
        conv1_tile(3)
        conv2_tile(2)
        conv1_tile(4)
        conv2_tile(3)

        # ---- per-block: softmax-normalize, scatter, DMA-transpose, reassemble
        for g in range(4):                     # output DMA groups of 4 blocks
            ostg0 = sbw.tile([128, 2048], f16, tag="ostg0", bufs=2)
            ostg1 = sbw.tile([128, 2048], f16, tag="ostg1", bufs=2)
            ostg = [ostg0, ostg1]
            for i in range(4):
                t = g * 4 + i
                q, qi = t // 4, t % 4
                # PE-transpose wk block -> (px, 100) in PSUM
                pwkT = ps.tile([128, ENC], f16, tag="pwkT", bufs=2, name=f"pwkT{t}")
                nc.tensor.transpose(pwkT[:], wkq[q][:, qi * 128:(qi + 1) * 128],
                                    ident[0:100, 0:100])
                # softmax over 25 taps: group sums, reciprocal, scale
                sums = sbw.tile([128, 4], f32, tag="sums", bufs=4, name=f"sums{t}")
                nc.vector.tensor_reduce(
                    out=sums[:],
                    in_=pwkT[:].rearrange("z (p k) -> z p k", k=25),
                    axis=mybir.AxisListType.X, op=mybir.AluOpType.add)
                recip = sbw.tile([128, 4], f32, tag="recip", bufs=4, name=f"recip{t}")
                nc.vector.reciprocal(recip[:], sums[:])
                wkN = sbw.tile([128, 100], f16, tag="wkN", bufs=6, name=f"wkN{t}")
                rb = bass.AP(tensor=recip.tensor, offset=recip.offset,
                             ap=[recip.ap[0], [1, 4], [0, 25]])
                nc.vector.tensor_mul(
                    wkN[:].rearrange("z (p k) -> z p k", k=25),
                    pwkT[:].rearrange("z (p k) -> z p k", k=25),
                    rb)

                # scatter into band-matrix transpose layout (p, jb_rel, rb, wi)
                sdst = sbw.tile([128, 1536], f16, tag="sdst", bufs=6, name=f"sdst{t}")
                nc.gpsimd.local_scatter(
                    out_ap=sdst[:], data_ap=wkN[:], idxs_ap=sidx[:],
                    channels=128, num_elems=1536, num_idxs=100)

                # DMA-XBAR transpose: all 12 [128,128] S panels in one shot;
                # descriptor generation is expensive, alternate SP/Act queues
                s16 = sbw.tile([128, 12, 128], f16, tag="s16", bufs=6, name=f"s16_{t}")
                nc.sync.dma_start(out=s16, in_=sdst[:], transpose=True)

                # reassembly: po[ch, (p, outpx)] += xt.T @ S
                for ch in range(2):
                    po = ps.tile([128, 512], f32, tag="po", bufs=3, name=f"po{t}_{ch}")
                    for dj in range(3):
                        rhs = bass.AP(tensor=s16.tensor,
                                      offset=s16.offset + dj * 128,
                                      ap=[s16.ap[0], [384, 4], [1, 128]])
                        nc.tensor.matmul(
                            po[:],
                            xt[:, (t + dj) * 256 + ch * 128:
                               (t + dj) * 256 + ch * 128 + 128],
                            rhs, start=(dj == 0), stop=(dj == 2))
                    # evict with (p, rt, w) -> (rt, w, p) interleave, fp16;
                    # split across Act (ch0) and DVE (ch1)
                    src = bass.AP(tensor=po.tensor, offset=po.offset,
                                  ap=[po.ap[0], [64, 2], [1, 64], [128, 4]])
                    dst = ostg[ch][:, i * 512:(i + 1) * 512].rearrange(
                        "c (a b d) -> c a b d", a=2, b=64)
                    nc.scalar.activation(
                        out=dst, in_=src,
                        func=mybir.ActivationFunctionType.Copy, scale=1.0)
            for ch in range(2):
                nc.sync.dma_start(
                    out=d_out[ch * 128:(ch + 1) * 128, g * 2048:(g + 1) * 2048],
                    in_=ostg[ch])

    nc.compile()
    return nc


def _host_prep(x, W_comp, b_comp, W_enc, b_enc):
    """Build per-core input maps."""
    idxs = _build_idxs()
    wcT = np.ascontiguousarray(W_comp.T).astype(np.float16)            # (256, 64)
    # we[m, tap, o] = W_enc[o, m, i, j], tap = 3i + j
    weT = np.ascontiguousarray(
        W_enc.transpose(1, 2, 3, 0).reshape(MID, 9 * ENC)).astype(np.float16)
    bc = np.ascontiguousarray(b_comp.reshape(MID, 1)).astype(np.float32)
    be = np.ascontiguousarray(b_enc.reshape(ENC, 1)).astype(np.float32)

    xp = np.pad(x, ((0, 0), (0, 0), (2, 2), (0, 0)))   # (B, C, 68, 64)
    in_maps = []
    for core in range(8):
        b, half = core // 2, core % 2
        r0 = 32 * half
        xs = np.ascontiguousarray(
            xp[b, :, r0:r0 + NROW, :].reshape(C, NPX)).astype(np.float16)
        xth = np.ascontiguousarray(
            xs.reshape(C, NJB, 128).transpose(2, 1, 0)).reshape(128, NJB * C)
        in_maps.append(dict(x=xs, xt=xth, wc=wcT, we=weT, bc=bc, be=be,
                            idx=idxs))
    return in_maps


def kernel(x, W_comp, b_comp, W_enc, b_enc):
    x = np.asarray(x, np.float32)
    W_comp = np.asarray(W_comp, np.float32)
    b_comp = np.asarray(b_comp, np.float32)
    W_enc = np.asarray(W_enc, np.float32)
    b_enc = np.asarray(b_enc, np.float32)

    if "nc" not in _CACHE:
        _CACHE["nc"] = _build_nc()
    nc = _CACHE["nc"]

    in_maps = _host_prep(x, W_comp, b_comp, W_enc, b_enc)
    res = run_bass_kernel_spmd(nc, in_maps, core_ids=list(range(8)))

    out = np.empty((B, C, 128, 128), np.float32)
    for core in range(8):
        b, half = core // 2, core % 2
        seg = res.results[core]["out"].astype(np.float32)   # (256, 8192)
        out[b, :, 64 * half:64 * (half + 1), :] = seg.reshape(C, 64, 128)
    return out


if __name__ == "__main__":
    rng = np.random.default_rng(0)
    x = rng.standard_normal((B, C, H, W)).astype(np.float32)
    W_comp = (rng.standard_normal((MID, C)) / np.sqrt(C)).astype(np.float32)
    b_comp = np.zeros((MID,), np.float32)
    W_enc = (rng.standard_normal((ENC, MID, 3, 3)) / np.sqrt(MID * 9)).astype(np.float32)
    b_enc = np.zeros((ENC,), np.float32)
    out = kernel(x, W_comp, b_comp, W_enc, b_enc)
    print("out", out.shape, out.dtype, float(np.abs(out).mean()))


# revision 35
# speedup vs baseline: 1.1964x; 1.0760x over previous
"""CARAFE++ content-aware upsampling kernel for Trainium2 (8 NeuronCores).

Problem: x (4, 256, 64, 64) f32; 1x1 compress conv (256->64) + relu;
3x3 encoder conv (64->100); softmax over 25 taps; content-aware reassembly
(5x5 dynamic per-pixel filter, scale 2); flat pixel rearrangement to
(4, 256, 128, 128).

Sharding: 8 cores = 4 batches x 2 row-halves (32 rows each + halo).
All compute per-core independent (no collectives).

Per-core pipeline (v2):
  - host sends x as fp16 in BOTH layouts: (ch, px) for the convs and
    pre-transposed (px, ch) for the reassembly (kills 36 PE transposes)
  - conv1 as 2 accumulated fp16 matmuls per col-tile; relu eviction written
    twice (partitions 0-63 = feat, 64-127 = feat shifted one row) so conv2
    can pair vertical taps into K=128 matmuls (9 -> 6 matmuls per tile)
  - conv2 + bias + exp -> wk fp16; DMA-XBAR transpose -> wkT (px-major)
  - per 128-px block: DVE tap-group sums + reciprocal + normalize (softmax),
    gpsimd local_scatter builds band-matrix-transpose S^T, one DMA-XBAR
    transpose turns it into 12 [128,128] S panels (src-major)
  - reassembly out = x_T.T @ S: 6 accumulated fp16 matmuls per block
  - interleaved fp16 eviction; batched DMA store; fp16 -> f32 on host
"""
import sys

sys.path.insert(0, "/opt/trn_rl_repo")

import numpy as np
from contextlib import ExitStack

import concourse.bass as bass
import concourse.bacc as bacc
import concourse.tile as tile
from concourse import mybir
from concourse.bass_utils import run_bass_kernel_spmd

B, C, H, W = 4, 256, 64, 64
SCALE, K, COMP, G = 2, 5, 4, 1
MID = 64
ENC = 100          # K*K*SCALE*SCALE
NROW = 36          # x rows per core (32 + 2 halo each side)
NPX = NROW * W     # 2304
FROW = 34          # feat rows (rows r0-1 .. r0+32)
FPW = W + 2        # 66, feat row W-padded
NBLK = 16          # output row-pair blocks per core
NJB = 18           # x row-pair blocks per core

f32 = mybir.dt.float32
f16 = mybir.dt.float16
i16 = mybir.dt.int16

_CACHE = {}

# x chunk column ranges (per 128-ch half): conv1 tile nt reads x cols
# 64+512*nt .. 64+512*(nt+1); chunk c covers tiles CHUNK_TILES[c]
CHUNKS = [(0, 1088), (1088, 2112), (2112, 2304)]
TILE_CHUNK = [0, 0, 1, 1, 2]


def _build_idxs():
    """Per-partition scatter indices encoding the CARAFE tap geometry.

    Partition = out-pixel (rt, w) within a row-pair block. Slot = (p, dy, dx)
    = wk channel order. Value = position in the (p, jb_rel, rb, wi) scatter
    destination, or -1 when the tap falls outside the image in W.
    """
    idxs = np.full((128, 100), -1, np.int16)
    for rt in range(2):
        for w in range(W):
            part = rt * W + w
            for p in range(4):
                for dy in range(-2, 3):
                    jb_rel = (rt + dy + 2) // 2      # 0..2
                    rb = (rt + dy) % 2
                    for dx in range(-2, 3):
                        wi = w + dx
                        if 0 <= wi < W:
                            slot = p * 25 + (dy + 2) * 5 + (dx + 2)
                            idxs[part, slot] = p * 384 + jb_rel * 128 + rb * 64 + wi
    return idxs


def _build_nc():
    nc = bacc.Bacc("TRN2", target_bir_lowering=False, debug=False, num_devices=8)

    # ---- DRAM I/O (per-core shapes)
    d_x = nc.dram_tensor("x", [C, NPX], f16, kind="ExternalInput")
    d_xt = nc.dram_tensor("xt", [128, NJB * 256], f16, kind="ExternalInput")
    d_wc = nc.dram_tensor("wc", [C, MID], f16, kind="ExternalInput")        # W_comp.T
    d_we = nc.dram_tensor("we", [MID, 9 * ENC], f16, kind="ExternalInput")
    d_bc = nc.dram_tensor("bc", [MID, 1], f32, kind="ExternalInput")
    d_be = nc.dram_tensor("be", [ENC, 1], f32, kind="ExternalInput")
    d_idx = nc.dram_tensor("idx", [128, 100], i16, kind="ExternalInput")
    d_out = nc.dram_tensor("out", [C, 32 * 256], f16, kind="ExternalOutput")

    with tile.TileContext(nc) as tc, ExitStack() as ctx:
        sb1 = ctx.enter_context(tc.tile_pool(name="sb1", bufs=1))
        sbw = ctx.enter_context(tc.tile_pool(name="sbw", bufs=2))
        ps = ctx.enter_context(tc.tile_pool(name="ps", bufs=2, space="PSUM"))

        # ---- input DMA dispatches split across SP and Act queues so conv1
        # can start ASAP (x chunk halves dispatched in parallel)
        wc0 = sb1.tile([128, MID], f16, tag="wc0")
        wc1 = sb1.tile([128, MID], f16, tag="wc1")
        bc = sb1.tile([MID, 1], f32, tag="bc")
        xc = [[None] * len(CHUNKS) for _ in range(2)]
        for k, (c0, c1) in enumerate(CHUNKS):
            for h in range(2):
                t = sbw.tile([128, c1 - c0], f16, tag=f"xc{h}{k}", bufs=1)
                xc[h][k] = t

        nc.sync.dma_start(out=xc[0][0], in_=d_x[0:128, 0:CHUNKS[0][1]])
        nc.scalar.dma_start(out=xc[1][0], in_=d_x[128:256, 0:CHUNKS[0][1]])
        nc.sync.dma_start(out=wc0, in_=d_wc[0:128, :])
        nc.scalar.dma_start(out=wc1, in_=d_wc[128:256, :])
        nc.sync.dma_start(out=bc, in_=d_bc[:])
        we = sb1.tile([MID, 9, ENC], f16, tag="we")
        nc.scalar.dma_start(out=we, in_=d_we[:].rearrange("m (t o) -> m t o", t=9))
        for k in range(1, len(CHUNKS)):
            c0, c1 = CHUNKS[k]
            nc.sync.dma_start(out=xc[0][k], in_=d_x[0:128, c0:c1])
            nc.scalar.dma_start(out=xc[1][k], in_=d_x[128:256, c0:c1])
        be = sb1.tile([ENC, 1], f32, tag="be")
        nc.scalar.dma_start(out=be, in_=d_be[:])
        sidx = sb1.tile([128, 100], i16, tag="sidx")
        nc.scalar.dma_start(out=sidx, in_=d_idx[:])
        xt = sb1.tile([128, NJB * 256], f16, tag="xt")
        nc.scalar.dma_start(out=xt, in_=d_xt[:])

        # identity for PE transposes
        ident = sb1.tile([128, 128], f16, tag="ident")
        nc.vector.memset(ident, 1.0)
        nc.gpsimd.affine_select(
            out=ident[:], in_=ident[:], pattern=[[-1, 128]], base=0,
            channel_multiplier=1, compare_op=mybir.AluOpType.is_equal, fill=0.0,
        )
        # ---- feat (W-padded, fp16) on partitions 0-63; zero pad cols
        featD = sb1.tile([MID, FROW * FPW], f16, tag="featD")
        pad = bass.AP(tensor=featD.tensor, offset=featD.offset,
                      ap=[featD.ap[0], [FPW, FROW], [FPW - 1, 2]])
        nc.vector.memset(pad, 0.0)

        # ---- conv1 (1x1, 256->64) + relu -> featD rows, interleaved with
        # conv2 (3x3, 64->100, 9 shifted accumulated matmuls) + bias + exp
        # -> wkq[q] [100, 512] fp16.
        wkq = [None] * 4

        def conv1_tile(nt):
            n0 = W + nt * 512          # px offset into x
            n = min(512, 2240 - n0)
            ck = TILE_CHUNK[nt]
            lo = n0 - CHUNKS[ck][0]
            pf = ps.tile([MID, 512], f32, tag="pf", bufs=1, name=f"pf{nt}")
            nc.tensor.matmul(pf[:, :n], wc0[:], xc[0][ck][:, lo:lo + n],
                             start=True, stop=False)
            nc.tensor.matmul(pf[:, :n], wc1[:], xc[1][ck][:, lo:lo + n],
                             start=False, stop=True)
            fp0 = n0 // W - 1
            nrows = n // W
            dst = bass.AP(
                tensor=featD.tensor, offset=featD.offset + fp0 * FPW + 1,
                ap=[featD.ap[0], [FPW, nrows], [1, W]],
            )
            nc.scalar.activation(out=dst, in_=pf[:, :n].rearrange("m (r w) -> m r w", w=W),
                                 func=mybir.ActivationFunctionType.Relu,
                                 bias=bc[:], scale=1.0)

        def conv2_tile(q):
            h0 = q * 8
            pw = ps.tile([ENC, 512], f32, tag="pw", name=f"pw{q}")
            for tap in range(9):
                i, j = tap // 3, tap % 3
                rhs = bass.AP(tensor=featD.tensor,
                              offset=featD.offset + (h0 + i) * FPW + j,
                              ap=[featD.ap[0], [FPW, 8], [1, W]])
                nc.tensor.matmul(pw[:], we[:, tap, :], rhs,
                                 start=(tap == 0), stop=(tap == 8))
            wt = sbw.tile([ENC, 512], f16, tag=f"wkq{q}", name=f"wkq{q}")
            nc.scalar.activation(out=wt[:], in_=pw[:],
                                 func=mybir.ActivationFunctionType.Exp,
                                 bias=be[:], scale=1.0)
            wkq[q] = wt

        conv1_tile(0)
        conv1_tile(1)
        conv2_tile(0)
        conv1_tile(2)
        conv2_tile(1)
        conv1_tile(3)
        conv2_tile(2)
        conv1_tile(4)
        conv2_tile(3)

        # ---- per-block: softmax-normalize, scatter, DMA-transpose, reassemble
        for g in range(4):                     # output DMA groups of 4 blocks
            ostg0 = sbw.tile([128, 2048], f16, tag="ostg0", bufs=2)
            ostg1 = sbw.tile([128, 2048], f16, tag="ostg1", bufs=2)
            ostg = [ostg0, ostg1]
            for i in range(4):
                t = g * 4 + i
                q, qi = t // 4, t % 4
                # PE-transpose wk block -> (px, 100) in PSUM
                pwkT = ps.tile([128, ENC], f16, tag="pwkT", bufs=2, name=f"pwkT{t}")
                nc.tensor.transpose(pwkT[:], wkq[q][:, qi * 128:(qi + 1) * 128],
                                    ident[0:100, 0:100])
                # softmax over 25 taps: group sums, reciprocal, scale
                sums = sbw.tile([128, 4], f32, tag="sums", bufs=4, name=f"sums{t}")
                nc.vector.tensor_reduce(
                    out=sums[:],
                    in_=pwkT[:].rearrange("z (p k) -> z p k", k=25),
                    axis=mybir.AxisListType.X, op=mybir.AluOpType.add)
                recip = sbw.tile([128, 4], f32, tag="recip", bufs=4, name=f"recip{t}")
                nc.vector.reciprocal(recip[:], sums[:])
                wkN = sbw.tile([128, 100], f16, tag="wkN", bufs=6, name=f"wkN{t}")
                rb = bass.AP(tensor=recip.tensor, offset=recip.offset,
                             ap=[recip.ap[0], [1, 4], [0, 25]])
                nc.vector.tensor_mul(
                    wkN[:].rearrange("z (p k) -> z p k", k=25),
                    pwkT[:].rearrange("z (p k) -> z p k", k=25),
                    rb)

                # scatter into band-matrix transpose layout (p, jb_rel, rb, wi)
                sdst = sbw.tile([128, 1536], f16, tag="sdst", bufs=6, name=f"sdst{t}")
                nc.gpsimd.local_scatter(
                    out_ap=sdst[:], data_ap=wkN[:], idxs_ap=sidx[:],
                    channels=128, num_elems=1536, num_idxs=100)

                # DMA-XBAR transpose: all 12 [128,128] S panels in one shot;
                # descriptor generation is expensive, alternate SP/Act queues
                s16 = sbw.tile([128, 12, 128], f16, tag="s16", bufs=6, name=f"s16_{t}")
                nc.sync.dma_start(out=s16, in_=sdst[:], transpose=True)

                # reassembly: po[ch, (p, outpx)] += xt.T @ S
                for ch in range(2):
                    po = ps.tile([128, 512], f32, tag="po", bufs=3, name=f"po{t}_{ch}")
                    for dj in range(3):
                        rhs = bass.AP(tensor=s16.tensor,
                                      offset=s16.offset + dj * 128,
                                      ap=[s16.ap[0], [384, 4], [1, 128]])
                        nc.tensor.matmul(
                            po[:],
                            xt[:, (t + dj) * 256 + ch * 128:
                               (t + dj) * 256 + ch * 128 + 128],
                            rhs, start=(dj == 0), stop=(dj == 2))
                    # evict with (p, rt, w) -> (rt, w, p) interleave, fp16;
                    # split across Act (ch0) and DVE (ch1)
                    src = bass.AP(tensor=po.tensor, offset=po.offset,
                                  ap=[po.ap[0], [64, 2], [1, 64], [128, 4]])
                    dst = ostg[ch][:, i * 512:(i + 1) * 512].rearrange(
                        "c (a b d) -> c a b d", a=2, b=64)
                    if ch == 0:
                        nc.scalar.activation(
                            out=dst, in_=src,
                            func=mybir.ActivationFunctionType.Copy, scale=1.0)
                    else:
                        nc.vector.tensor_copy(dst, src)
            for ch in range(2):
                nc.scalar.dma_start(
                    out=d_out[ch * 128:(ch + 1) * 128, g * 2048:(g + 1) * 2048],
                    in_=ostg[ch])

    nc.compile()
    return nc


def _host_prep(x, W_comp, b_comp, W_enc, b_enc):
    """Build per-core input maps."""
    idxs = _build_idxs()
    wcT = np.ascontiguousarray(W_comp.T).astype(np.float16)            # (256, 64)
    # we[m, tap, o] = W_enc[o, m, i, j], tap = 3i + j
    weT = np.ascontiguousarray(
        W_enc.transpose(1, 2, 3, 0).reshape(MID, 9 * ENC)).astype(np.float16)
    bc = np.ascontiguousarray(b_comp.reshape(MID, 1)).astype(np.float32)
    be = np.ascontiguousarray(b_enc.reshape(ENC, 1)).astype(np.float32)

    xp = np.pad(x, ((0, 0), (0, 0), (2, 2), (0, 0)))   # (B, C, 68, 64)
    in_maps = []
    for core in range(8):
        b, half = core // 2, core % 2
        r0 = 32 * half
        xs = np.ascontiguousarray(
            xp[b, :, r0:r0 + NROW, :].reshape(C, NPX)).astype(np.float16)
        xth = np.ascontiguousarray(
            xs.reshape(C, NJB, 128).transpose(2, 1, 0)).reshape(128, NJB * C)
        in_maps.append(dict(x=xs, xt=xth, wc=wcT, we=weT, bc=bc, be=be,
                            idx=idxs))
    return in_maps


def kernel(x, W_comp, b_comp, W_enc, b_enc):
    x = np.asarray(x, np.float32)
    W_comp = np.asarray(W_comp, np.float32)
    b_comp = np.asarray(b_comp, np.float32)
    W_enc = np.asarray(W_enc, np.float32)
    b_enc = np.asarray(b_enc, np.float32)

    if "nc" not in _CACHE:
        _CACHE["nc"] = _build_nc()
    nc = _CACHE["nc"]

    in_maps = _host_prep(x, W_comp, b_comp, W_enc, b_enc)
    res = run_bass_kernel_spmd(nc, in_maps, core_ids=list(range(8)))

    out = np.empty((B, C, 128, 128), np.float32)
    for core in range(8):
        b, half = core // 2, core % 2
        seg = res.results[core]["out"].astype(np.float32)   # (256, 8192)
        out[b, :, 64 * half:64 * (half + 1), :] = seg.reshape(C, 64, 128)
    return out


if __name__ == "__main__":
    rng = np.random.default_rng(0)
    x = rng.standard_normal((B, C, H, W)).astype(np.float32)
    W_comp = (rng.standard_normal((MID, C)) / np.sqrt(C)).astype(np.float32)
    b_comp = np.zeros((MID,), np.float32)
    W_enc = (rng.standard_normal((ENC, MID, 3, 3)) / np.sqrt(MID * 9)).astype(np.float32)
    b_enc = np.zeros((ENC,), np.float32)
    out = kernel(x, W_comp, b_comp, W_enc, b_enc)
    print("out", out.shape, out.dtype, float(np.abs(out).mean()))
